# revision 17
# baseline (speedup 1.0000x reference)
"""Causal attention (QKV proj + softmax(QK^T/sqrt(d))V) on 8 TRN2 NeuronCores.

Sharding: data-parallel over batch (B=8, one batch element per core).
Per-core kernel, all matmuls in float32r (fast fp32 mode):
  phase 0:  x [T,D] -> x^T [D,T] via PE transposes (SBUF resident)
  phase 1a: V -> DRAM scratch (so its reload can overlap phase 1b)
  phase 1b: Q^T -> DRAM scratch; K^T -> SBUF resident; V half 0 and the
            first supertile's Q tiles reload into non-reused SBUF behind it
  phase 2:  per 512-wide query supertile: S^T = K Q^T (so softmax probs are
            produced directly in the lhsT layout needed by P@V), exp on ACT
            with fused 1/sqrt(D) scale, causal mask on the diagonal band,
            P@V + ones-matmul row sums on PE, reciprocal normalize, store.

DMA rings: sync = phase-0/1 streaming loads; scalar = stores;
gpsimd (SWDGE) = odd x tiles + prefetch reloads (qt/V), so prefetches
issue at dependency-readiness instead of queueing behind blocked rings.
"""

import numpy as np

T = 2048
D = 1024
E = 1024
N_CORES = 8
P = 128
TS = 512  # t-slice / supertile width
SCALE = 1.0 / 32.0  # 1/sqrt(D)

DC = D // P  # 8 d-chunks
EC = E // P  # 8 e-chunks
TB = T // P  # 16 t-blocks of 128
NTS = T // TS  # 4 t-slices of 512
JB = TS // P  # 4 q-blocks per supertile


def _attention_kernel(ctx, tc, out, x, wq, wk, wv):
    import concourse.bass as bass
    from concourse import mybir
    from concourse.bass import ts
    from concourse.masks import make_identity

    nc = tc.nc
    f32 = mybir.dt.float32
    f32r = mybir.dt.float32r
    AF = mybir.ActivationFunctionType

    # ---- DRAM scratch ----
    dram = ctx.enter_context(tc.tile_pool(name="dram", bufs=1, space="DRAM"))
    qdram = dram.tile([EC, P, T], f32r)  # Q^T[e,t], e = ec*128 + ep
    vdram = dram.tile([TB, P, E], f32r)  # V[t,e], t = tb*128 + tp

    # ---- left-side SBUF pools (never overlap reused space) ----
    const = ctx.enter_context(tc.tile_pool(name="const", bufs=1))
    ones_f32 = const.tile([P, 2], f32)
    nc.vector.memset(ones_f32[:], 1.0)
    ones_col = const.tile([P, 2], f32r)
    nc.vector.tensor_copy(ones_col[:], ones_f32[:])
    identity_f32 = const.tile([P, P], f32)
    make_identity(nc, identity_f32[:])
    identity = const.tile([P, P], f32r)
    nc.vector.tensor_copy(identity[:], identity_f32[:])

    kt_pool = ctx.enter_context(tc.tile_pool(name="ktres", bufs=1))
    KT = kt_pool.tile([P, EC, T], f32r)  # K^T[e, t], e = ec*128 + ep

    # One wide causal mask; mask_j = wide[:, 384-128j : 384-128j+512] keeps
    # entries where f - p - 128*j >= 0 in S^T coords (p=key, f=query).
    mask_pool = ctx.enter_context(tc.tile_pool(name="maskp", bufs=1))
    WIDE = TS + (JB - 1) * P
    wide_f32 = mask_pool.tile([P, WIDE], f32)
    nc.gpsimd.memset(wide_f32[:], 1.0)
    nc.gpsimd.affine_select(
        out=wide_f32[:],
        in_=wide_f32[:],
        compare_op=mybir.AluOpType.is_ge,
        fill=0.0,
        base=-(JB - 1) * P,
        pattern=[[1, WIDE]],
        channel_multiplier=-1,
    )
    wide = mask_pool.tile([P, WIDE], f32r)
    nc.vector.tensor_copy(wide[:], wide_f32[:])
    masks = [
        wide[:, (JB - 1) * P - P * j : (JB - 1) * P - P * j + TS]
        for j in range(JB)
    ]

    # ---- right-side work pools for phases 0/1 ----
    tc.swap_default_side()
    xt_pool = tc.alloc_tile_pool(name="xt", bufs=1)
    xT = xt_pool.tile([P, DC, T], f32r)  # x^T[d, t], d = dc*128 + dp
    qstg = tc.alloc_tile_pool(name="qstg", bufs=3)
    wqk_pool = tc.alloc_tile_pool(name="wqk", bufs=2)
    wvh_pool = tc.alloc_tile_pool(name="wvh", bufs=2)
    tc.swap_default_side()

    # ---- PSUM pools for phases 0/1 ----
    ps_tp = tc.alloc_tile_pool(name="ps_tp", bufs=2, space="PSUM")
    ps_proj = tc.alloc_tile_pool(name="ps_proj", bufs=4, space="PSUM")

    # ===== phase 0: x -> x^T via PE transposes (f32r: 1.5 cyc/row) =====
    for tb in range(TB):
        xa = qstg.tile([P, D], f32r, tag="qstage")
        eng = nc.sync if tb % 2 == 0 else nc.gpsimd
        eng.dma_start(xa[:, 0 : D // 2], x[ts(tb, P), 0 : D // 2].bitcast(f32r))
        eng.dma_start(xa[:, D // 2 : D], x[ts(tb, P), D // 2 : D].bitcast(f32r))
        for dc in range(DC):
            pt = ps_tp.tile([P, P], f32r)
            nc.tensor.transpose(pt[:], xa[:, ts(dc, P)], identity[:])
            # evict + round to f32r; alternate engines
            if dc % 2 == 0:
                nc.vector.tensor_copy(xT[:, dc, ts(tb, P)], pt[:])
            else:
                nc.scalar.copy(xT[:, dc, ts(tb, P)], pt[:])

    # ============ phase 1a: V = x @ Wv (to DRAM scratch) ============
    wv_view = wv.bitcast(f32r).rearrange("(dc dp) e -> dp dc e", dp=P)
    for eh in range(E // TS):
        wvh = wvh_pool.tile([P, DC, TS], f32r, tag="wvh")
        nc.sync.dma_start(wvh[:], wv_view[:, :, ts(eh, TS)])
        for tb in range(TB):
            pp = ps_proj.tile([P, TS], f32)
            for dc in range(DC):
                nc.tensor.matmul(
                    pp[:],
                    xT[:, dc, ts(tb, P)],
                    wvh[:, dc, :],
                    start=(dc == 0),
                    stop=(dc == DC - 1),
                )
            vst = qstg.tile([P, TS], f32r, tag="qstage")
            nc.scalar.copy(vst[:], pp[:])
            nc.scalar.dma_start(vdram[tb, :, ts(eh, TS)], vst[:])

    wvh_pool.release()

    # ======== phase 1b: Q^T (to DRAM scratch), K^T (resident) ========
    # Prefetch targets for phase 2 (left side: usable before x^T dies).
    qt0_pool = ctx.enter_context(tc.tile_pool(name="qt0", bufs=EC))
    v_pool0 = ctx.enter_context(tc.tile_pool(name="vres0", bufs=1))
    # V half 0 reloads into SBUF behind the Q/K projections (SWDGE ring).
    Vh0 = v_pool0.tile([P, TB, TS], f32r)
    nc.gpsimd.dma_start(
        Vh0[:], vdram[:, :, ts(0, TS)].rearrange("tb p e -> p tb e")
    )

    qts0 = []
    for w_ap, is_q in ((wq, True), (wk, False)):
        w_view = w_ap.bitcast(f32r).rearrange("(dc dp) e -> dp dc e", dp=P)
        for eb in range(EC):
            wr = wqk_pool.tile([P, DC, P], f32r, tag="wqk")
            nc.sync.dma_start(wr[:], w_view[:, :, ts(eb, P)])
            for tsl in range(NTS):
                pp = ps_proj.tile([P, TS], f32)
                for dc in range(DC):
                    nc.tensor.matmul(
                        pp[:],
                        wr[:, dc, :],
                        xT[:, dc, ts(tsl, TS)],
                        start=(dc == 0),
                        stop=(dc == DC - 1),
                    )
                if is_q:
                    qst = qstg.tile([P, TS], f32r, tag="qstage")
                    nc.scalar.copy(qst[:], pp[:])
                    nc.scalar.dma_start(qdram[eb, :, ts(tsl, TS)], qst[:])
                else:
                    nc.scalar.copy(KT[:, eb, ts(tsl, TS)], pp[:])
        if is_q:
            # prefetch supertile 0's Q tiles while K projects (SWDGE ring)
            for ec in range(EC):
                q1 = qt0_pool.tile([P, TS], f32r, tag="qt0")
                nc.gpsimd.dma_start(q1[:], qdram[ec, :, ts(0, TS)])
                qts0.append(q1)

    wqk_pool.release()
    qstg.release()
    xt_pool.release()
    ps_proj.release()
    ps_tp.release()

    # ================= phase 2: attention =================
    v_pool1 = ctx.enter_context(tc.tile_pool(name="vres1", bufs=1))
    Vh1 = v_pool1.tile([P, TB, TS], f32r)
    nc.gpsimd.dma_start(
        Vh1[:], vdram[:, :, ts(1, TS)].rearrange("tb p e -> p tb e")
    )
    Vh = [Vh0, Vh1]

    ps_s = tc.alloc_tile_pool(name="ps_s", bufs=3, space="PSUM")
    ps_o = tc.alloc_tile_pool(name="ps_o", bufs=2, space="PSUM")
    ps_sum = tc.alloc_tile_pool(name="ps_sum", bufs=2, space="PSUM")

    tc.swap_default_side()
    qt_pool = ctx.enter_context(tc.tile_pool(name="qt", bufs=8))
    pt_pool = ctx.enter_context(tc.tile_pool(name="pt", bufs=1))
    rs_pool = ctx.enter_context(tc.tile_pool(name="rs", bufs=4))
    ostg = ctx.enter_context(tc.tile_pool(name="ostg", bufs=3))
    tc.swap_default_side()

    for sup in range(NTS):
        nkb = JB * sup + JB  # key blocks 0..nkb-1
        if sup == 0:
            qts = qts0
        else:
            qts = []
            for ec in range(EC):
                q1 = qt_pool.tile([P, TS], f32r, tag="qt")
                nc.gpsimd.dma_start(q1[:], qdram[ec, :, ts(sup, TS)])
                qts.append(q1)
        pT = pt_pool.tile([P, TB, TS], f32r)

        # --- S^T blocks + exp + causal mask ---
        for k in range(nkb):
            ssp = ps_s.tile([P, TS], f32)
            for ec in range(EC):
                nc.tensor.matmul(
                    ssp[:],
                    KT[:, ec, ts(k, P)],
                    qts[ec][:],
                    start=(ec == 0),
                    stop=(ec == EC - 1),
                )
            nc.scalar.activation(pT[:, k, :], ssp[:], AF.Exp, scale=SCALE)
            j = k - JB * sup
            if j >= 0:
                nc.vector.tensor_mul(pT[:, k, :], pT[:, k, :], masks[j])

        # --- row sums, then P @ V per e-half (eh-outer: Vh1 gets time
        # to land while eh=0 is computed), normalize, store ---
        rss = []
        for jq in range(JB):
            qb = JB * sup + jq
            nk = qb + 1
            pos = ps_sum.tile([P, 2], f32)
            for k in range(nk):
                nc.tensor.matmul(
                    pos[:],
                    pT[:, k, ts(jq, P)],
                    ones_col[:],
                    start=(k == 0),
                    stop=(k == nk - 1),
                )
            rs = rs_pool.tile([P, 1], f32)
            nc.vector.reciprocal(rs[:], pos[:, 0:1])
            rss.append(rs)
        for eh in range(E // TS):
            for jq in range(JB):
                qb = JB * sup + jq
                nk = qb + 1
                po = ps_o.tile([P, TS], f32)
                for k in range(nk):
                    nc.tensor.matmul(
                        po[:],
                        pT[:, k, ts(jq, P)],
                        Vh[eh][:, k, :],
                        start=(k == 0),
                        stop=(k == nk - 1),
                    )
                ost = ostg.tile([P, TS], f32, tag="ostage")
                nc.vector.tensor_scalar_mul(ost[:], po[:], rss[jq][:])
                nc.scalar.dma_start(out[ts(qb, P), ts(eh, TS)], ost[:])

    ps_sum.release()
    ps_o.release()
    ps_s.release()


def build_program():
    from contextlib import ExitStack

    import concourse.bacc as bacc
    import concourse.tile as tile
    from concourse import mybir

    nc = bacc.Bacc("TRN2", target_bir_lowering=False, debug=False)
    f32 = mybir.dt.float32
    x = nc.dram_tensor("x", [T, D], f32, kind="ExternalInput").ap()
    wq = nc.dram_tensor("Wq", [D, E], f32, kind="ExternalInput").ap()
    wk = nc.dram_tensor("Wk", [D, E], f32, kind="ExternalInput").ap()
    wv = nc.dram_tensor("Wv", [D, E], f32, kind="ExternalInput").ap()
    out = nc.dram_tensor("out", [T, E], f32, kind="ExternalOutput").ap()

    with tile.TileContext(nc) as tc:
        with ExitStack() as ctx:
            _attention_kernel(ctx, tc, out, x, wq, wk, wv)
    nc.compile()
    return nc


def kernel(x, Wq, Wk, Wv, _trace=False):
    from concourse.bass_utils import run_bass_kernel_spmd

    x = np.ascontiguousarray(np.asarray(x), dtype=np.float32)
    Wq = np.ascontiguousarray(np.asarray(Wq), dtype=np.float32)
    Wk = np.ascontiguousarray(np.asarray(Wk), dtype=np.float32)
    Wv = np.ascontiguousarray(np.asarray(Wv), dtype=np.float32)
    assert x.shape == (N_CORES, T, D), x.shape

    nc = build_program()
    in_maps = [
        {"x": np.ascontiguousarray(x[b]), "Wq": Wq, "Wk": Wk, "Wv": Wv}
        for b in range(N_CORES)
    ]
    res = run_bass_kernel_spmd(
        nc, in_maps, core_ids=list(range(N_CORES)), trace=_trace
    )
    out = np.stack([res.results[b]["out"] for b in range(N_CORES)], axis=0)
    if _trace:
        kernel.last_results = res
    return out


kernel.last_results = None


# revision 18
# speedup vs baseline: 1.0160x; 1.0160x over previous
"""Causal attention (QKV proj + softmax(QK^T/sqrt(d))V) on 8 TRN2 NeuronCores.

Sharding: data-parallel over batch (B=8, one batch element per core).
Per-core kernel, all matmuls in float32r (fast fp32 mode):
  phase 0:  x [T,D] -> x^T [D,T] via PE transposes (SBUF resident)
  phase 1a: V -> DRAM scratch (so its reload can overlap phase 1b)
  phase 1b: Q^T -> DRAM scratch; K^T -> SBUF resident; V half 0 and the
            first supertile's Q tiles reload into non-reused SBUF behind it
  phase 2:  per 512-wide query supertile: S^T = K Q^T (so softmax probs are
            produced directly in the lhsT layout needed by P@V), exp on ACT
            with fused 1/sqrt(D) scale, causal mask on the diagonal band,
            P@V + ones-matmul row sums on PE, reciprocal normalize, store.

DMA rings: sync = phase-0/1 streaming loads; scalar = stores;
gpsimd (SWDGE) = odd x tiles + prefetch reloads (qt/V), so prefetches
issue at dependency-readiness instead of queueing behind blocked rings.
"""

import numpy as np

T = 2048
D = 1024
E = 1024
N_CORES = 8
P = 128
TS = 512  # t-slice / supertile width
SCALE = 1.0 / 32.0  # 1/sqrt(D)

DC = D // P  # 8 d-chunks
EC = E // P  # 8 e-chunks
TB = T // P  # 16 t-blocks of 128
NTS = T // TS  # 4 t-slices of 512
JB = TS // P  # 4 q-blocks per supertile


def _attention_kernel(ctx, tc, out, x, wq, wk, wv):
    import concourse.bass as bass
    from concourse import mybir
    from concourse.bass import ts
    from concourse.masks import make_identity

    nc = tc.nc
    f32 = mybir.dt.float32
    f32r = mybir.dt.float32r
    AF = mybir.ActivationFunctionType

    # ---- DRAM scratch ----
    dram = ctx.enter_context(tc.tile_pool(name="dram", bufs=1, space="DRAM"))
    qdram = dram.tile([EC, P, T], f32r)  # Q^T[e,t], e = ec*128 + ep
    vdram = dram.tile([TB, P, E], f32r)  # V[t,e], t = tb*128 + tp

    # ---- left-side SBUF pools (never overlap reused space) ----
    const = ctx.enter_context(tc.tile_pool(name="const", bufs=1))
    ones_f32 = const.tile([P, 2], f32)
    nc.vector.memset(ones_f32[:], 1.0)
    ones_col = const.tile([P, 2], f32r)
    nc.vector.tensor_copy(ones_col[:], ones_f32[:])
    identity_f32 = const.tile([P, P], f32)
    make_identity(nc, identity_f32[:])
    identity = const.tile([P, P], f32r)
    nc.vector.tensor_copy(identity[:], identity_f32[:])

    kt_pool = ctx.enter_context(tc.tile_pool(name="ktres", bufs=1))
    KT = kt_pool.tile([P, EC, T], f32r)  # K^T[e, t], e = ec*128 + ep

    # One wide causal mask; mask_j = wide[:, 384-128j : 384-128j+512] keeps
    # entries where f - p - 128*j >= 0 in S^T coords (p=key, f=query).
    mask_pool = ctx.enter_context(tc.tile_pool(name="maskp", bufs=1))
    WIDE = TS + (JB - 1) * P
    wide_f32 = mask_pool.tile([P, WIDE], f32)
    nc.gpsimd.memset(wide_f32[:], 1.0)
    nc.gpsimd.affine_select(
        out=wide_f32[:],
        in_=wide_f32[:],
        compare_op=mybir.AluOpType.is_ge,
        fill=0.0,
        base=-(JB - 1) * P,
        pattern=[[1, WIDE]],
        channel_multiplier=-1,
    )
    wide = mask_pool.tile([P, WIDE], f32r)
    nc.vector.tensor_copy(wide[:], wide_f32[:])
    masks = [
        wide[:, (JB - 1) * P - P * j : (JB - 1) * P - P * j + TS]
        for j in range(JB)
    ]

    # ---- right-side work pools for phases 0/1 ----
    tc.swap_default_side()
    xt_pool = tc.alloc_tile_pool(name="xt", bufs=1)
    xT = xt_pool.tile([P, DC, T], f32r)  # x^T[d, t], d = dc*128 + dp
    qstg = tc.alloc_tile_pool(name="qstg", bufs=3)
    wqk_pool = tc.alloc_tile_pool(name="wqk", bufs=2)
    wvh_pool = tc.alloc_tile_pool(name="wvh", bufs=2)
    tc.swap_default_side()

    # ---- PSUM pools for phases 0/1 ----
    ps_tp = tc.alloc_tile_pool(name="ps_tp", bufs=2, space="PSUM")
    ps_proj = tc.alloc_tile_pool(name="ps_proj", bufs=4, space="PSUM")

    # ===== phase 0: x -> x^T via PE transposes (f32r: 1.5 cyc/row) =====
    for tb in range(TB):
        xa = qstg.tile([P, D], f32r, tag="qstage")
        eng = nc.sync if tb % 2 == 0 else nc.gpsimd
        eng.dma_start(xa[:, 0 : D // 2], x[ts(tb, P), 0 : D // 2].bitcast(f32r))
        eng.dma_start(xa[:, D // 2 : D], x[ts(tb, P), D // 2 : D].bitcast(f32r))
        for dc in range(DC):
            pt = ps_tp.tile([P, P], f32r)
            nc.tensor.transpose(pt[:], xa[:, ts(dc, P)], identity[:])
            # evict + round to f32r; alternate engines
            if dc % 2 == 0:
                nc.vector.tensor_copy(xT[:, dc, ts(tb, P)], pt[:])
            else:
                nc.scalar.copy(xT[:, dc, ts(tb, P)], pt[:])

    # ============ phase 1a: V = x @ Wv (to DRAM scratch) ============
    wv_view = wv.bitcast(f32r).rearrange("(dc dp) e -> dp dc e", dp=P)
    for eh in range(E // TS):
        wvh = wvh_pool.tile([P, DC, TS], f32r, tag="wvh")
        nc.sync.dma_start(wvh[:], wv_view[:, :, ts(eh, TS)])
        for tb in range(TB):
            pp = ps_proj.tile([P, TS], f32)
            for dc in range(DC):
                nc.tensor.matmul(
                    pp[:],
                    xT[:, dc, ts(tb, P)],
                    wvh[:, dc, :],
                    start=(dc == 0),
                    stop=(dc == DC - 1),
                )
            vst = qstg.tile([P, TS], f32r, tag="qstage")
            nc.vector.tensor_copy(vst[:], pp[:])
            nc.scalar.dma_start(vdram[tb, :, ts(eh, TS)], vst[:])

    wvh_pool.release()

    # ======== phase 1b: Q^T (to DRAM scratch), K^T (resident) ========
    # Prefetch targets for phase 2 (left side: usable before x^T dies).
    qt0_pool = ctx.enter_context(tc.tile_pool(name="qt0", bufs=EC))
    v_pool0 = ctx.enter_context(tc.tile_pool(name="vres0", bufs=1))
    # V half 0 reloads into SBUF behind the Q/K projections (SWDGE ring).
    Vh0 = v_pool0.tile([P, TB, TS], f32r)
    nc.gpsimd.dma_start(
        Vh0[:], vdram[:, :, ts(0, TS)].rearrange("tb p e -> p tb e")
    )

    qts0 = []
    for w_ap, is_q in ((wq, True), (wk, False)):
        w_view = w_ap.bitcast(f32r).rearrange("(dc dp) e -> dp dc e", dp=P)
        for eb in range(EC):
            wr = wqk_pool.tile([P, DC, P], f32r, tag="wqk")
            nc.sync.dma_start(wr[:], w_view[:, :, ts(eb, P)])
            for tsl in range(NTS):
                pp = ps_proj.tile([P, TS], f32)
                for dc in range(DC):
                    nc.tensor.matmul(
                        pp[:],
                        wr[:, dc, :],
                        xT[:, dc, ts(tsl, TS)],
                        start=(dc == 0),
                        stop=(dc == DC - 1),
                    )
                if is_q:
                    qst = qstg.tile([P, TS], f32r, tag="qstage")
                    nc.vector.tensor_copy(qst[:], pp[:])
                    nc.scalar.dma_start(qdram[eb, :, ts(tsl, TS)], qst[:])
                else:
                    nc.vector.tensor_copy(KT[:, eb, ts(tsl, TS)], pp[:])
        if is_q:
            # prefetch supertile 0's Q tiles while K projects (SWDGE ring)
            for ec in range(EC):
                q1 = qt0_pool.tile([P, TS], f32r, tag="qt0")
                nc.gpsimd.dma_start(q1[:], qdram[ec, :, ts(0, TS)])
                qts0.append(q1)

    wqk_pool.release()
    qstg.release()
    xt_pool.release()
    ps_proj.release()
    ps_tp.release()

    # ================= phase 2: attention =================
    v_pool1 = ctx.enter_context(tc.tile_pool(name="vres1", bufs=1))
    Vh1 = v_pool1.tile([P, TB, TS], f32r)
    nc.gpsimd.dma_start(
        Vh1[:], vdram[:, :, ts(1, TS)].rearrange("tb p e -> p tb e")
    )
    Vh = [Vh0, Vh1]

    ps_s = tc.alloc_tile_pool(name="ps_s", bufs=4, space="PSUM")
    ps_o = tc.alloc_tile_pool(name="ps_o", bufs=2, space="PSUM")
    ps_sum = tc.alloc_tile_pool(name="ps_sum", bufs=2, space="PSUM")

    tc.swap_default_side()
    qt_pool = ctx.enter_context(tc.tile_pool(name="qt", bufs=8))
    pt_pool = ctx.enter_context(tc.tile_pool(name="pt", bufs=1))
    rs_pool = ctx.enter_context(tc.tile_pool(name="rs", bufs=4))
    ostg = ctx.enter_context(tc.tile_pool(name="ostg", bufs=3))
    tc.swap_default_side()

    for sup in range(NTS):
        nkb = JB * sup + JB  # key blocks 0..nkb-1
        if sup == 0:
            qts = qts0
        else:
            qts = []
            for ec in range(EC):
                q1 = qt_pool.tile([P, TS], f32r, tag="qt")
                nc.gpsimd.dma_start(q1[:], qdram[ec, :, ts(sup, TS)])
                qts.append(q1)
        pT = pt_pool.tile([P, TB, TS], f32r)

        # --- S^T blocks + exp + causal mask ---
        for k in range(nkb):
            ssp = ps_s.tile([P, TS], f32)
            for ec in range(EC):
                nc.tensor.matmul(
                    ssp[:],
                    KT[:, ec, ts(k, P)],
                    qts[ec][:],
                    start=(ec == 0),
                    stop=(ec == EC - 1),
                )
            nc.scalar.activation(pT[:, k, :], ssp[:], AF.Exp, scale=SCALE)
            j = k - JB * sup
            if j >= 0:
                nc.vector.tensor_mul(pT[:, k, :], pT[:, k, :], masks[j])

        # --- row sums, then P @ V per e-half (eh-outer: Vh1 gets time
        # to land while eh=0 is computed), normalize, store ---
        rss = []
        for jq in range(JB):
            qb = JB * sup + jq
            nk = qb + 1
            pos = ps_sum.tile([P, 2], f32)
            for k in range(nk):
                nc.tensor.matmul(
                    pos[:],
                    pT[:, k, ts(jq, P)],
                    ones_col[:],
                    start=(k == 0),
                    stop=(k == nk - 1),
                )
            rs = rs_pool.tile([P, 1], f32)
            nc.vector.reciprocal(rs[:], pos[:, 0:1])
            rss.append(rs)
        for eh in range(E // TS):
            for jq in range(JB):
                qb = JB * sup + jq
                nk = qb + 1
                po = ps_o.tile([P, TS], f32)
                for k in range(nk):
                    nc.tensor.matmul(
                        po[:],
                        pT[:, k, ts(jq, P)],
                        Vh[eh][:, k, :],
                        start=(k == 0),
                        stop=(k == nk - 1),
                    )
                ost = ostg.tile([P, TS], f32, tag="ostage")
                nc.vector.tensor_scalar_mul(ost[:], po[:], rss[jq][:])
                nc.scalar.dma_start(out[ts(qb, P), ts(eh, TS)], ost[:])

    ps_sum.release()
    ps_o.release()
    ps_s.release()


def build_program():
    from contextlib import ExitStack

    import concourse.bacc as bacc
    import concourse.tile as tile
    from concourse import mybir

    nc = bacc.Bacc("TRN2", target_bir_lowering=False, debug=False)
    f32 = mybir.dt.float32
    x = nc.dram_tensor("x", [T, D], f32, kind="ExternalInput").ap()
    wq = nc.dram_tensor("Wq", [D, E], f32, kind="ExternalInput").ap()
    wk = nc.dram_tensor("Wk", [D, E], f32, kind="ExternalInput").ap()
    wv = nc.dram_tensor("Wv", [D, E], f32, kind="ExternalInput").ap()
    out = nc.dram_tensor("out", [T, E], f32, kind="ExternalOutput").ap()

    with tile.TileContext(nc) as tc:
        with ExitStack() as ctx:
            _attention_kernel(ctx, tc, out, x, wq, wk, wv)
    nc.compile()
    return nc


def kernel(x, Wq, Wk, Wv, _trace=False):
    from concourse.bass_utils import run_bass_kernel_spmd

    x = np.ascontiguousarray(np.asarray(x), dtype=np.float32)
    Wq = np.ascontiguousarray(np.asarray(Wq), dtype=np.float32)
    Wk = np.ascontiguousarray(np.asarray(Wk), dtype=np.float32)
    Wv = np.ascontiguousarray(np.asarray(Wv), dtype=np.float32)
    assert x.shape == (N_CORES, T, D), x.shape

    nc = build_program()
    in_maps = [
        {"x": np.ascontiguousarray(x[b]), "Wq": Wq, "Wk": Wk, "Wv": Wv}
        for b in range(N_CORES)
    ]
    res = run_bass_kernel_spmd(
        nc, in_maps, core_ids=list(range(N_CORES)), trace=_trace
    )
    out = np.stack([res.results[b]["out"] for b in range(N_CORES)], axis=0)
    if _trace:
        kernel.last_results = res
    return out


kernel.last_results = None
